# revision 33
# baseline (speedup 1.0000x reference)
"""Trainium2 Bass kernel for 16-head causal MHA (B=4, S=2048, D=1024).

Sharding: 8 cores = 4 batches x 2 head-groups (8 heads each).
Each core computes, for its batch b and head-group g:
  qh_T = (Wq_g @ x_q^T + bq_g) / 8   [512 j, S]   (j = head-major features)
  kh_T =  Wk_g @ x_k^T + bk_g        [512 j, S]
  vh   =  x_v @ Wv_g^T + bv_g        [S, 512 j]   (+ ones column per head)
  per head h (64-dim): block-causal scores_T [k, q], exp (no max
  subtraction; scores are O(1)), diagonal-block masking, ctx_T[dh,q]
  (+ sumexp row via the ones column), batched-reciprocal normalize,
  y_partial = ctx_T^T @ Wo_g^T       [S, 1024]
Host sums the two head-group partials per batch and adds bo.

Matmuls run in bf16 (fp32 PSUM accumulation); set MM_DT to float32r for
a higher-precision (but ~2.5x slower) variant.
"""

import os
import sys
from contextlib import ExitStack

sys.path.insert(0, "/opt/trn_rl_repo")

import numpy as np
import ml_dtypes

import concourse.bass as bass
import concourse.tile as tile
from concourse import mybir
from concourse.bass_utils import run_bass_kernel_spmd

F32 = mybir.dt.float32
MM_DT = mybir.dt.bfloat16
NP_MM = ml_dtypes.bfloat16
P = 128

# Full-problem dims
B, S_FULL, D, H = 4, 2048, 1024, 16
DH = D // H          # 64
G = 512              # features per head-group (8 heads x 64)
HLOC = 8             # heads per core

LAST_RESULTS = None  # set by kernel() for test harness introspection
RUN_KWARGS = {}  # extra kwargs for run_bass_kernel_spmd (test harness only)


def _mm(nc, out, lhsT, rhs, **kw):
    nc.tensor.matmul(out, lhsT, rhs, **kw)


_FIX_TYPES = {
    "InstMatmult",
    "InstActivation",
    "InstTensorCopy",
    "InstTensorTensor",
    "InstTensorScalarPtr",
    "InstTensorReduce",
    "InstReciprocal",
    "InstMemset",
    "InstDMACopy",
    "InstDrain",
    "InstEventSemaphore",
}


def _fixup_matmul_waits(nc):
    """walrus packs an instruction's sem waits into a single ISA struct with
    room for only ONE wait command (observed for Matmult/LW and Activation).
    Hoist excess waits onto same-engine nops inserted immediately before the
    offending instruction (same-engine order preserves the happens-before)."""
    import bass_rust as _br

    f = nc.m.functions[0]
    blocks = list(f.blocks)
    per_engine = {}
    for b in blocks:
        for inst in b.instructions:
            if type(inst).__name__ in _FIX_TYPES:
                si = inst.sync_info
                if si is not None and si.on_wait and len(si.on_wait) > 1:
                    eng = str(inst.engine)
                    per_engine[eng] = per_engine.get(eng, 0) + len(si.on_wait) - 1
    if not per_engine:
        return
    eng_map = {
        "EngineType.PE": nc.tensor,
        "EngineType.Activation": nc.scalar,
        "EngineType.DVE": nc.vector,
        "EngineType.Pool": nc.gpsimd,
        "EngineType.SP": nc.sync,
    }
    nop_stash = {}
    for eng, cnt in per_engine.items():
        lens_before = {b.name: len(b.instructions) for b in blocks}
        handle = eng_map[eng]
        for _ in range(cnt):
            handle.nop()
        endb = next(b for b in blocks if len(b.instructions) != lens_before[b.name])
        endlist = endb.instructions
        nop_stash[eng] = list(endlist[-cnt:])
        endb.instructions = endlist[:-cnt]
    for b in blocks:
        insts = b.instructions
        out = []
        changed = False
        for inst in insts:
            if type(inst).__name__ in _FIX_TYPES:
                si = inst.sync_info
                if si is not None and si.on_wait and len(si.on_wait) > 1:
                    waits = list(si.on_wait)
                    eng = str(inst.engine)
                    for w in waits[:-1]:
                        nop = nop_stash[eng].pop(0)
                        nop.sync_info = _br.SyncInfo(on_wait=[w], on_update=[])
                        out.append(nop)
                    si.on_wait = [waits[-1]]
                    changed = True
            out.append(inst)
        if changed:
            b.instructions = out


def build_program(S=S_FULL):
    """Build the single-core Bass program (same program on all 8 cores)."""
    ND = D // P           # 8 d-tiles
    NJ = G // P           # 4 j-tiles
    NST = S // P          # s-tiles
    NCH = S // 512        # q-chunks (and s-quarters)

    nc = bass.Bass("TRN2", target_bir_lowering=False, debug=False)

    xqT = nc.dram_tensor("xqT", [ND, NCH, P, 512], MM_DT, kind="ExternalInput").ap()
    xkT = nc.dram_tensor("xkT", [ND, NCH, P, 512], MM_DT, kind="ExternalInput").ap()
    xvT = nc.dram_tensor("xvT", [ND, NCH, P, 512], MM_DT, kind="ExternalInput").ap()
    wqT = nc.dram_tensor("wqT", [P, ND, G], MM_DT, kind="ExternalInput").ap()
    wkT = nc.dram_tensor("wkT", [P, ND, G], MM_DT, kind="ExternalInput").ap()
    wvT = nc.dram_tensor("wvT", [P, ND, G], MM_DT, kind="ExternalInput").ap()
    woT = nc.dram_tensor("woT", [P, NJ, D], MM_DT, kind="ExternalInput").ap()
    bqd = nc.dram_tensor("bq", [G], F32, kind="ExternalInput").ap()
    bkd = nc.dram_tensor("bk", [G], F32, kind="ExternalInput").ap()
    bvd = nc.dram_tensor("bv", [G], MM_DT, kind="ExternalInput").ap()
    mkd = nc.dram_tensor("maskb", [P, 4, 512], MM_DT, kind="ExternalInput").ap()
    idd = nc.dram_tensor("ident", [P, P], MM_DT, kind="ExternalInput").ap()
    yp = nc.dram_tensor("yp", [S, D], F32, kind="ExternalOutput").ap()

    with tile.TileContext(nc) as tc, ExitStack() as ctx:
        consts = ctx.enter_context(tc.tile_pool(name="consts", bufs=1))
        big = ctx.enter_context(tc.tile_pool(name="big", bufs=1))
        wpool = ctx.enter_context(tc.tile_pool(name="weights", bufs=4))
        xpool = ctx.enter_context(tc.tile_pool(name="xt", bufs=48))
        apool = ctx.enter_context(tc.tile_pool(name="attn", bufs=4))
        opool = ctx.enter_context(tc.tile_pool(name="outev", bufs=2))
        rpool = ctx.enter_context(tc.tile_pool(name="recip", bufs=2))
        pp = ctx.enter_context(tc.tile_pool(name="pp", bufs=2, space="PSUM"))
        sp = ctx.enter_context(tc.tile_pool(name="sp", bufs=2, space="PSUM"))
        cp = ctx.enter_context(tc.tile_pool(name="cp", bufs=2, space="PSUM"))

        # --- constants ---
        ones1f = consts.tile([1, P], F32)
        nc.vector.memset(ones1f, 1.0)
        ones1 = consts.tile([1, P], MM_DT)
        nc.vector.tensor_copy(ones1, ones1f)
        bvrow = consts.tile([1, G], MM_DT)
        nc.sync.dma_start(bvrow, bvd.rearrange("(o g) -> o g", o=1))
        bqc = consts.tile([P, NJ], F32)
        nc.sync.dma_start(bqc, bqd.rearrange("(a p) -> p a", a=NJ))
        bkc = consts.tile([P, NJ], F32)
        nc.sync.dma_start(bkc, bkd.rearrange("(a p) -> p a", a=NJ))
        maskb = consts.tile([P, 4, 512], MM_DT)
        nc.sync.dma_start(maskb, mkd)
        ident = consts.tile([P, P], MM_DT)
        nc.sync.dma_start(ident, idd)

        # --- persistent activations ---
        QT = big.tile([P, NJ, S], MM_DT)
        KT = big.tile([P, NJ, S], MM_DT)
        CT = big.tile([P, NJ, S], MM_DT)   # normalized ctx_T for the out-proj
        V = big.tile([P, NST, HLOC, DH + 1], MM_DT)
        onesvf = consts.tile([P, NST * HLOC], F32)
        nc.vector.memset(onesvf, 1.0)
        nc.vector.tensor_copy(V[:, :, :, DH : DH + 1], onesvf)

        # --- weights (wv first: v-proj runs first; rest after v-proj x DMAs) ---
        wv_sb = wpool.tile([P, ND, G], MM_DT, tag="w")
        nc.sync.dma_start(wv_sb, wvT)

        # --- q/k projections, jt-at-a-time so attention can interleave ---
        def load_x(x_dram):
            xs = {}
            for sq in range(NCH):
                for dt in range(ND):
                    xt = xpool.tile([P, 512], MM_DT, tag="x")
                    nc.sync.dma_start(xt, x_dram[dt, sq])
                    xs[dt, sq] = xt
            return xs

        def proj_jt(xs, w_sb, dst, bias_sb, scale, jt):
            for sq in range(NCH):
                ps = pp.tile([P, 512], F32, tag="pp")
                for dt in range(ND):
                    _mm(
                        nc, ps,
                        w_sb[:, dt, jt * P : (jt + 1) * P],
                        xs[dt, sq],
                        start=(dt == 0), stop=(dt == ND - 1),
                    )
                nc.scalar.activation(
                    dst[:, jt, sq * 512 : (sq + 1) * 512], ps,
                    mybir.ActivationFunctionType.Identity,
                    bias=bias_sb[:, jt : jt + 1], scale=scale,
                )

        # --- v projection first: out [s-part, j-free], ones col per head ---
        for sq in range(NCH):
            xs = []
            for dt in range(ND):
                xt = xpool.tile([P, 512], MM_DT, tag="x")
                nc.sync.dma_start(xt, xvT[dt, sq])
                xs.append(xt)
            for stl in range(4):
                st = sq * 4 + stl
                ps = pp.tile([P, G], F32, tag="pp")
                for dt in range(ND):
                    _mm(
                        nc, ps,
                        xs[dt][:, stl * P : (stl + 1) * P],
                        wv_sb[:, dt, :],
                        start=(dt == 0), stop=False,
                    )
                _mm(nc, ps, ones1, bvrow, start=False, stop=True)
                nc.vector.tensor_copy(
                    V[:, st, :, 0:DH], ps.rearrange("p (h d) -> p h d", h=HLOC)
                )
            if sq == 0:
                wk_sb = wpool.tile([P, ND, G], MM_DT, tag="w")
                nc.sync.dma_start(wk_sb, wkT)
                wq_sb = wpool.tile([P, ND, G], MM_DT, tag="w")
                nc.sync.dma_start(wq_sb, wqT)
                wo_sb = wpool.tile([P, NJ, D], MM_DT, tag="w")
                nc.sync.dma_start(wo_sb, woT)

        xs_k = load_x(xkT)
        for jt in range(NJ):
            proj_jt(xs_k, wk_sb, KT, bkc, 1.0, jt)
        xs_q = load_x(xqT)
        proj_jt(xs_q, wq_sb, QT, bqc, 0.125, 0)

        # --- attention: head pairs interleaved at chunk level ---
        pending_norm = None
        for t in range(HLOC // 2):
            heads = (2 * t, 2 * t + 1)
            sums = {}
            for h in heads:
                sums_t = rpool.tile([P, 512], F32, tag=f"sums{h % 2}")
                nc.vector.memset(sums_t, 1.0)
                sums[h] = sums_t
            for c in range(NCH):
                cs = slice(c * 512, (c + 1) * 512)
                hA, hB = heads
                jtp = t
                ctxA = cp.tile([P, 512], F32, tag="cp")
                ctxB = cp.tile([P, 512], F32, tag="cp")
                nkt = 4 * (c + 1)
                for kt in range(nkt):
                    diag = kt >= nkt - 4
                    scp = sp.tile([P, 2, 512], F32, tag="sp")
                    _mm(
                        nc, scp[:, 0, :],
                        KT[0:DH, jtp, kt * P : (kt + 1) * P],
                        QT[0:DH, jtp, cs],
                        start=True, stop=not diag,
                    )
                    _mm(
                        nc, scp[:, 1, :],
                        KT[DH:P, jtp, kt * P : (kt + 1) * P],
                        QT[DH:P, jtp, cs],
                        start=True, stop=not diag,
                    )
                    if diag:  # accumulate -1e9 above the diagonal, pre-exp
                        moff = kt - (nkt - 4)
                        _mm(
                            nc, scp[:, 0, :], ident, maskb[:, moff, :],
                            start=False, stop=True,
                        )
                        _mm(
                            nc, scp[:, 1, :], ident, maskb[:, moff, :],
                            start=False, stop=True,
                        )
                    at = apool.tile([P, 2, 512], MM_DT, tag="attn")
                    nc.scalar.activation(
                        at, scp, mybir.ActivationFunctionType.Exp
                    )
                    _mm(
                        nc, ctxA[0 : DH + 1, :],
                        V[:, kt, hA, :], at[:, 0, :],
                        start=(kt == 0), stop=(kt == nkt - 1),
                    )
                    _mm(
                        nc, ctxB[0 : DH + 1, :],
                        V[:, kt, hB, :], at[:, 1, :],
                        start=(kt == 0), stop=(kt == nkt - 1),
                    )
                # evict unnormalized ctx into CT; stash the sumexp rows
                nc.vector.tensor_copy(CT[0:DH, jtp, cs], ctxA[0:DH, :])
                nc.vector.tensor_copy(
                    sums[hA][32 * c : 32 * c + 1, :], ctxA[DH : DH + 1, :]
                )
                nc.vector.tensor_copy(CT[DH:P, jtp, cs], ctxB[0:DH, :])
                nc.vector.tensor_copy(
                    sums[hB][32 * c : 32 * c + 1, :], ctxB[DH : DH + 1, :]
                )
                if c == min(2, NCH - 1) and pending_norm is not None:
                    pending_norm()
                    pending_norm = None
            # start DVE reciprocal chains now; defer broadcast+mul to next pair
            norm_parts = []
            for h in heads:
                jt, half = h // 2, (h % 2) * DH
                rec_h = rpool.tile([P, 512], MM_DT, tag=f"rech{h % 2}")
                with nc.allow_low_precision(reason="softmax recip"):
                    nc.vector.reciprocal(rec_h, sums[h])
                rstages = []
                for c in range(NCH):
                    rstage = rpool.tile([1, 512], MM_DT, tag=f"rst{h % 2}_{c}")
                    nc.vector.tensor_copy(rstage, rec_h[32 * c : 32 * c + 1, :])
                    rstages.append(rstage)
                norm_parts.append((jt, half, rstages))

            def _make_norm(parts=norm_parts):
                def _norm():
                    for jt, half, rstages in parts:
                        for c in range(NCH):
                            cs = slice(c * 512, (c + 1) * 512)
                            rbp = pp.tile([P, 512], F32, tag="pp")
                            _mm(
                                nc, rbp[0:DH, :], ones1[:, 0:DH], rstages[c],
                                start=True, stop=True,
                            )
                            rb = rpool.tile([P, 512], MM_DT, tag="rb")
                            nc.vector.tensor_copy(
                                rb[half : half + DH, :], rbp[0:DH, :]
                            )
                            nc.vector.tensor_mul(
                                CT[half : half + DH, jt, cs],
                                CT[half : half + DH, jt, cs],
                                rb[half : half + DH, :],
                            )
                return _norm

            pending_norm = _make_norm()
            if t + 1 < NJ:
                proj_jt(xs_q, wq_sb, QT, bqc, 0.125, t + 1)
        if pending_norm is not None:
            pending_norm()
            pending_norm = None

        # --- output projection: y[s, e] = ctx_T^T @ woT ---
        for st in range(NST):
            ob = opool.tile([P, D], F32, tag="ob")
            for ec in range(2):
                ps = pp.tile([P, 512], F32, tag="pp")
                for jtt in range(NJ):
                    _mm(
                        nc, ps,
                        CT[:, jtt, st * P : (st + 1) * P],
                        wo_sb[:, jtt, ec * 512 : (ec + 1) * 512],
                        start=(jtt == 0), stop=(jtt == NJ - 1),
                    )
                nc.vector.tensor_copy(ob[:, ec * 512 : (ec + 1) * 512], ps)
            nc.sync.dma_start(yp[st * P : (st + 1) * P, :], ob)

    _fixup_matmul_waits(nc)
    return nc


def _arr_x(x, S):
    """[S, D] -> [ND, NCH, P, 512] bf16, contiguous DMA tiles of x^T."""
    xT = x.T.astype(NP_MM)  # [D, S]
    return np.ascontiguousarray(
        xT.reshape(D // P, P, S // 512, 512).transpose(0, 2, 1, 3)
    )


def _arr_w(wT):
    """[K, N] (K mult of 128) -> [P, K//P, N] bf16 partition-major tiles."""
    wT = np.asarray(wT).astype(NP_MM)
    kk, n = wT.shape
    return np.ascontiguousarray(wT.reshape(kk // P, P, n).transpose(1, 0, 2))


def make_core_inputs(q, k, v, mask, wq, bq, wk, bk, wv, bv, wo, bo, S=S_FULL):
    """Build the 8 per-core input dicts. Core c -> batch c//2, head-group c%2."""
    f = np.float32
    # block-causal diagonal mask tiles [kk, j, qq] = mask[q0+qq, q0+128j+kk]
    m = np.asarray(mask)[0, 0]
    base = S - 512
    kk = np.arange(P)
    j = np.arange(4)
    qq = np.arange(512)
    mt = m[
        (base + qq)[None, None, :].repeat(P, 0).repeat(4, 1),
        (base + 128 * j[None, :, None] + kk[:, None, None]).repeat(512, 2),
    ]
    mb = np.where(mt != 0, np.float32(0.0), np.float32(-1e9)).astype(NP_MM)
    ident = np.eye(P, dtype=NP_MM)

    in_maps = []
    for c in range(8):
        b, g = c // 2, c % 2
        sl = slice(g * G, (g + 1) * G)
        in_maps.append(
            {
                "xqT": _arr_x(np.asarray(q[b]), S),
                "xkT": _arr_x(np.asarray(k[b]), S),
                "xvT": _arr_x(np.asarray(v[b]), S),
                "wqT": _arr_w(np.asarray(wq)[sl, :].T),
                "wkT": _arr_w(np.asarray(wk)[sl, :].T),
                "wvT": _arr_w(np.asarray(wv)[sl, :].T),
                "woT": _arr_w(np.asarray(wo)[:, sl].T),
                "bq": (np.asarray(bq)[sl] * 0.125).astype(f),
                "bk": np.asarray(bk)[sl].astype(f),
                "bv": np.asarray(bv)[sl].astype(NP_MM),
                "maskb": mb,
                "ident": ident,
            }
        )
    return in_maps


def kernel(q, k, v, mask, wq, bq, wk, bk, wv, bv, wo, bo):
    global LAST_RESULTS
    nc = build_program(S_FULL)
    in_maps = make_core_inputs(q, k, v, mask, wq, bq, wk, bk, wv, bv, wo, bo)
    res = run_bass_kernel_spmd(nc, in_maps, core_ids=list(range(8)), **RUN_KWARGS)
    LAST_RESULTS = res
    out = np.empty((B, S_FULL, D), np.float32)
    bo32 = np.asarray(bo, np.float32)
    for b in range(B):
        out[b] = res.results[2 * b]["yp"] + res.results[2 * b + 1]["yp"] + bo32
    return out


# revision 34
# speedup vs baseline: 1.0241x; 1.0241x over previous
"""Trainium2 Bass kernel for 16-head causal MHA (B=4, S=2048, D=1024).

Sharding: 8 cores = 4 batches x 2 head-groups (8 heads each).
Each core computes, for its batch b and head-group g:
  qh_T = (Wq_g @ x_q^T + bq_g) / 8   [512 j, S]   (j = head-major features)
  kh_T =  Wk_g @ x_k^T + bk_g        [512 j, S]
  vh   =  x_v @ Wv_g^T + bv_g        [S, 512 j]   (+ ones column per head)
  per head h (64-dim): block-causal scores_T [k, q], exp (no max
  subtraction; scores are O(1)), diagonal-block masking, ctx_T[dh,q]
  (+ sumexp row via the ones column), batched-reciprocal normalize,
  y_partial = ctx_T^T @ Wo_g^T       [S, 1024]
Host sums the two head-group partials per batch and adds bo.

Matmuls run in bf16 (fp32 PSUM accumulation); set MM_DT to float32r for
a higher-precision (but ~2.5x slower) variant.
"""

import os
import sys
from contextlib import ExitStack

sys.path.insert(0, "/opt/trn_rl_repo")

import numpy as np
import ml_dtypes

import concourse.bass as bass
import concourse.tile as tile
from concourse import mybir
from concourse.bass_utils import run_bass_kernel_spmd

F32 = mybir.dt.float32
MM_DT = mybir.dt.bfloat16
NP_MM = ml_dtypes.bfloat16
P = 128

# Full-problem dims
B, S_FULL, D, H = 4, 2048, 1024, 16
DH = D // H          # 64
G = 512              # features per head-group (8 heads x 64)
HLOC = 8             # heads per core

LAST_RESULTS = None  # set by kernel() for test harness introspection
RUN_KWARGS = {}  # extra kwargs for run_bass_kernel_spmd (test harness only)


def _mm(nc, out, lhsT, rhs, **kw):
    nc.tensor.matmul(out, lhsT, rhs, **kw)


_FIX_TYPES = {
    "InstMatmult",
    "InstActivation",
    "InstTensorCopy",
    "InstTensorTensor",
    "InstTensorScalarPtr",
    "InstTensorReduce",
    "InstReciprocal",
    "InstMemset",
    "InstDMACopy",
    "InstDrain",
    "InstEventSemaphore",
}


def _fixup_matmul_waits(nc):
    """walrus packs an instruction's sem waits into a single ISA struct with
    room for only ONE wait command (observed for Matmult/LW and Activation).
    Hoist excess waits onto same-engine nops inserted immediately before the
    offending instruction (same-engine order preserves the happens-before)."""
    import bass_rust as _br

    f = nc.m.functions[0]
    blocks = list(f.blocks)
    per_engine = {}
    for b in blocks:
        for inst in b.instructions:
            if type(inst).__name__ in _FIX_TYPES:
                si = inst.sync_info
                if si is not None and si.on_wait and len(si.on_wait) > 1:
                    eng = str(inst.engine)
                    per_engine[eng] = per_engine.get(eng, 0) + len(si.on_wait) - 1
    if not per_engine:
        return
    eng_map = {
        "EngineType.PE": nc.tensor,
        "EngineType.Activation": nc.scalar,
        "EngineType.DVE": nc.vector,
        "EngineType.Pool": nc.gpsimd,
        "EngineType.SP": nc.sync,
    }
    nop_stash = {}
    for eng, cnt in per_engine.items():
        lens_before = {b.name: len(b.instructions) for b in blocks}
        handle = eng_map[eng]
        for _ in range(cnt):
            handle.nop()
        endb = next(b for b in blocks if len(b.instructions) != lens_before[b.name])
        endlist = endb.instructions
        nop_stash[eng] = list(endlist[-cnt:])
        endb.instructions = endlist[:-cnt]
    for b in blocks:
        insts = b.instructions
        out = []
        changed = False
        for inst in insts:
            if type(inst).__name__ in _FIX_TYPES:
                si = inst.sync_info
                if si is not None and si.on_wait and len(si.on_wait) > 1:
                    waits = list(si.on_wait)
                    eng = str(inst.engine)
                    for w in waits[:-1]:
                        nop = nop_stash[eng].pop(0)
                        nop.sync_info = _br.SyncInfo(on_wait=[w], on_update=[])
                        out.append(nop)
                    si.on_wait = [waits[-1]]
                    changed = True
            out.append(inst)
        if changed:
            b.instructions = out


def build_program(S=S_FULL):
    """Build the single-core Bass program (same program on all 8 cores)."""
    ND = D // P           # 8 d-tiles
    NJ = G // P           # 4 j-tiles
    NST = S // P          # s-tiles
    NCH = S // 512        # q-chunks (and s-quarters)

    nc = bass.Bass("TRN2", target_bir_lowering=False, debug=False)

    xqT = nc.dram_tensor("xqT", [ND, NCH, P, 512], MM_DT, kind="ExternalInput").ap()
    xkT = nc.dram_tensor("xkT", [ND, NCH, P, 512], MM_DT, kind="ExternalInput").ap()
    xvT = nc.dram_tensor("xvT", [ND, NCH, P, 512], MM_DT, kind="ExternalInput").ap()
    wqT = nc.dram_tensor("wqT", [P, ND, G], MM_DT, kind="ExternalInput").ap()
    wkT = nc.dram_tensor("wkT", [P, ND, G], MM_DT, kind="ExternalInput").ap()
    wvT = nc.dram_tensor("wvT", [P, ND, G], MM_DT, kind="ExternalInput").ap()
    woT = nc.dram_tensor("woT", [P, NJ, D], MM_DT, kind="ExternalInput").ap()
    bqd = nc.dram_tensor("bq", [G], F32, kind="ExternalInput").ap()
    bkd = nc.dram_tensor("bk", [G], F32, kind="ExternalInput").ap()
    bvd = nc.dram_tensor("bv", [G], MM_DT, kind="ExternalInput").ap()
    mkd = nc.dram_tensor("maskb", [P, 4, 512], MM_DT, kind="ExternalInput").ap()
    idd = nc.dram_tensor("ident", [P, P], MM_DT, kind="ExternalInput").ap()
    yp = nc.dram_tensor("yp", [S, D], F32, kind="ExternalOutput").ap()

    with tile.TileContext(nc) as tc, ExitStack() as ctx:
        consts = ctx.enter_context(tc.tile_pool(name="consts", bufs=1))
        big = ctx.enter_context(tc.tile_pool(name="big", bufs=1))
        wpool = ctx.enter_context(tc.tile_pool(name="weights", bufs=4))
        xpool = ctx.enter_context(tc.tile_pool(name="xt", bufs=48))
        apool = ctx.enter_context(tc.tile_pool(name="attn", bufs=4))
        opool = ctx.enter_context(tc.tile_pool(name="outev", bufs=2))
        rpool = ctx.enter_context(tc.tile_pool(name="recip", bufs=2))
        pp = ctx.enter_context(tc.tile_pool(name="pp", bufs=2, space="PSUM"))
        sp = ctx.enter_context(tc.tile_pool(name="sp", bufs=2, space="PSUM"))
        cp = ctx.enter_context(tc.tile_pool(name="cp", bufs=2, space="PSUM"))

        # --- constants ---
        ones1f = consts.tile([1, P], F32)
        nc.vector.memset(ones1f, 1.0)
        ones1 = consts.tile([1, P], MM_DT)
        nc.vector.tensor_copy(ones1, ones1f)
        bvrow = consts.tile([1, G], MM_DT)
        nc.sync.dma_start(bvrow, bvd.rearrange("(o g) -> o g", o=1))
        bqc = consts.tile([P, NJ], F32)
        nc.sync.dma_start(bqc, bqd.rearrange("(a p) -> p a", a=NJ))
        bkc = consts.tile([P, NJ], F32)
        nc.sync.dma_start(bkc, bkd.rearrange("(a p) -> p a", a=NJ))
        maskb = consts.tile([P, 4, 512], MM_DT)
        nc.sync.dma_start(maskb, mkd)
        ident = consts.tile([P, P], MM_DT)
        nc.sync.dma_start(ident, idd)

        # --- persistent activations ---
        QT = big.tile([P, NJ, S], MM_DT)
        KT = big.tile([P, NJ, S], MM_DT)
        CT = big.tile([P, NJ, S], MM_DT)   # normalized ctx_T for the out-proj
        V = big.tile([P, NST, HLOC, DH + 1], MM_DT)
        onesvf = consts.tile([P, NST * HLOC], F32)
        nc.vector.memset(onesvf, 1.0)
        nc.vector.tensor_copy(V[:, :, :, DH : DH + 1], onesvf)

        # --- weights (wv first: v-proj runs first; rest after v-proj x DMAs) ---
        wv_sb = wpool.tile([P, ND, G], MM_DT, tag="w")
        nc.sync.dma_start(wv_sb, wvT)

        # --- q/k projections, jt-at-a-time so attention can interleave ---
        def load_x(x_dram):
            xs = {}
            for sq in range(NCH):
                for dt in range(ND):
                    xt = xpool.tile([P, 512], MM_DT, tag="x")
                    nc.sync.dma_start(xt, x_dram[dt, sq])
                    xs[dt, sq] = xt
            return xs

        def proj_jt(xs, w_sb, dst, bias_sb, scale, jt):
            for sq in range(NCH):
                ps = pp.tile([P, 512], F32, tag="pp")
                for dt in range(ND):
                    _mm(
                        nc, ps,
                        w_sb[:, dt, jt * P : (jt + 1) * P],
                        xs[dt, sq],
                        start=(dt == 0), stop=(dt == ND - 1),
                    )
                nc.vector.tensor_scalar(
                    dst[:, jt, sq * 512 : (sq + 1) * 512], ps,
                    scale, bias_sb[:, jt : jt + 1],
                    mybir.AluOpType.mult, mybir.AluOpType.add,
                )

        # --- v projection first: out [s-part, j-free], ones col per head ---
        for sq in range(NCH):
            xs = []
            for dt in range(ND):
                xt = xpool.tile([P, 512], MM_DT, tag="x")
                nc.sync.dma_start(xt, xvT[dt, sq])
                xs.append(xt)
            for stl in range(4):
                st = sq * 4 + stl
                ps = pp.tile([P, G], F32, tag="pp")
                for dt in range(ND):
                    _mm(
                        nc, ps,
                        xs[dt][:, stl * P : (stl + 1) * P],
                        wv_sb[:, dt, :],
                        start=(dt == 0), stop=False,
                    )
                _mm(nc, ps, ones1, bvrow, start=False, stop=True)
                nc.vector.tensor_copy(
                    V[:, st, :, 0:DH], ps.rearrange("p (h d) -> p h d", h=HLOC)
                )
            if sq == 0:
                wk_sb = wpool.tile([P, ND, G], MM_DT, tag="w")
                nc.sync.dma_start(wk_sb, wkT)
                wq_sb = wpool.tile([P, ND, G], MM_DT, tag="w")
                nc.sync.dma_start(wq_sb, wqT)
                wo_sb = wpool.tile([P, NJ, D], MM_DT, tag="w")
                nc.sync.dma_start(wo_sb, woT)

        xs_k = load_x(xkT)
        for jt in range(NJ):
            proj_jt(xs_k, wk_sb, KT, bkc, 1.0, jt)
        xs_q = load_x(xqT)
        proj_jt(xs_q, wq_sb, QT, bqc, 0.125, 0)

        # --- attention: head pairs interleaved at chunk level ---
        pending_norm = None
        for t in range(HLOC // 2):
            heads = (2 * t, 2 * t + 1)
            sums = {}
            for h in heads:
                sums_t = rpool.tile([P, 512], F32, tag=f"sums{h % 2}")
                nc.vector.memset(sums_t, 1.0)
                sums[h] = sums_t
            for c in range(NCH):
                cs = slice(c * 512, (c + 1) * 512)
                hA, hB = heads
                jtp = t
                ctxA = cp.tile([P, 512], F32, tag="cp")
                ctxB = cp.tile([P, 512], F32, tag="cp")
                nkt = 4 * (c + 1)
                for kt in range(nkt):
                    diag = kt >= nkt - 4
                    scp = sp.tile([P, 2, 512], F32, tag="sp")
                    _mm(
                        nc, scp[:, 0, :],
                        KT[0:DH, jtp, kt * P : (kt + 1) * P],
                        QT[0:DH, jtp, cs],
                        start=True, stop=not diag,
                    )
                    _mm(
                        nc, scp[:, 1, :],
                        KT[DH:P, jtp, kt * P : (kt + 1) * P],
                        QT[DH:P, jtp, cs],
                        start=True, stop=not diag,
                    )
                    if diag:  # accumulate -1e9 above the diagonal, pre-exp
                        moff = kt - (nkt - 4)
                        _mm(
                            nc, scp[:, 0, :], ident, maskb[:, moff, :],
                            start=False, stop=True,
                        )
                        _mm(
                            nc, scp[:, 1, :], ident, maskb[:, moff, :],
                            start=False, stop=True,
                        )
                    at = apool.tile([P, 2, 512], MM_DT, tag="attn")
                    nc.scalar.activation(
                        at, scp, mybir.ActivationFunctionType.Exp
                    )
                    _mm(
                        nc, ctxA[0 : DH + 1, :],
                        V[:, kt, hA, :], at[:, 0, :],
                        start=(kt == 0), stop=(kt == nkt - 1),
                    )
                    _mm(
                        nc, ctxB[0 : DH + 1, :],
                        V[:, kt, hB, :], at[:, 1, :],
                        start=(kt == 0), stop=(kt == nkt - 1),
                    )
                # evict unnormalized ctx into CT; stash the sumexp rows
                nc.vector.tensor_copy(CT[0:DH, jtp, cs], ctxA[0:DH, :])
                nc.vector.tensor_copy(
                    sums[hA][32 * c : 32 * c + 1, :], ctxA[DH : DH + 1, :]
                )
                nc.vector.tensor_copy(CT[DH:P, jtp, cs], ctxB[0:DH, :])
                nc.vector.tensor_copy(
                    sums[hB][32 * c : 32 * c + 1, :], ctxB[DH : DH + 1, :]
                )
                if c == min(2, NCH - 1) and pending_norm is not None:
                    pending_norm()
                    pending_norm = None
            # start DVE reciprocal chains now; defer broadcast+mul to next pair
            norm_parts = []
            for h in heads:
                jt, half = h // 2, (h % 2) * DH
                rec_h = rpool.tile([P, 512], MM_DT, tag=f"rech{h % 2}")
                with nc.allow_low_precision(reason="softmax recip"):
                    nc.vector.reciprocal(rec_h, sums[h])
                rstages = []
                for c in range(NCH):
                    rstage = rpool.tile([1, 512], MM_DT, tag=f"rst{h % 2}_{c}")
                    nc.vector.tensor_copy(rstage, rec_h[32 * c : 32 * c + 1, :])
                    rstages.append(rstage)
                norm_parts.append((jt, half, rstages))

            def _make_norm(parts=norm_parts):
                def _norm():
                    for jt, half, rstages in parts:
                        for c in range(NCH):
                            cs = slice(c * 512, (c + 1) * 512)
                            rbp = pp.tile([P, 512], F32, tag="pp")
                            _mm(
                                nc, rbp[0:DH, :], ones1[:, 0:DH], rstages[c],
                                start=True, stop=True,
                            )
                            rb = rpool.tile([P, 512], MM_DT, tag="rb")
                            nc.vector.tensor_copy(
                                rb[half : half + DH, :], rbp[0:DH, :]
                            )
                            nc.vector.tensor_mul(
                                CT[half : half + DH, jt, cs],
                                CT[half : half + DH, jt, cs],
                                rb[half : half + DH, :],
                            )
                return _norm

            pending_norm = _make_norm()
            if t + 1 < NJ:
                proj_jt(xs_q, wq_sb, QT, bqc, 0.125, t + 1)
        if pending_norm is not None:
            pending_norm()
            pending_norm = None

        # --- output projection: y[s, e] = ctx_T^T @ woT ---
        for st in range(NST):
            ob = opool.tile([P, D], F32, tag="ob")
            for ec in range(2):
                ps = pp.tile([P, 512], F32, tag="pp")
                for jtt in range(NJ):
                    _mm(
                        nc, ps,
                        CT[:, jtt, st * P : (st + 1) * P],
                        wo_sb[:, jtt, ec * 512 : (ec + 1) * 512],
                        start=(jtt == 0), stop=(jtt == NJ - 1),
                    )
                nc.vector.tensor_copy(ob[:, ec * 512 : (ec + 1) * 512], ps)
            nc.sync.dma_start(yp[st * P : (st + 1) * P, :], ob)

    _fixup_matmul_waits(nc)
    return nc


def _arr_x(x, S):
    """[S, D] -> [ND, NCH, P, 512] bf16, contiguous DMA tiles of x^T."""
    xT = x.T.astype(NP_MM)  # [D, S]
    return np.ascontiguousarray(
        xT.reshape(D // P, P, S // 512, 512).transpose(0, 2, 1, 3)
    )


def _arr_w(wT):
    """[K, N] (K mult of 128) -> [P, K//P, N] bf16 partition-major tiles."""
    wT = np.asarray(wT).astype(NP_MM)
    kk, n = wT.shape
    return np.ascontiguousarray(wT.reshape(kk // P, P, n).transpose(1, 0, 2))


def make_core_inputs(q, k, v, mask, wq, bq, wk, bk, wv, bv, wo, bo, S=S_FULL):
    """Build the 8 per-core input dicts. Core c -> batch c//2, head-group c%2."""
    f = np.float32
    # block-causal diagonal mask tiles [kk, j, qq] = mask[q0+qq, q0+128j+kk]
    m = np.asarray(mask)[0, 0]
    base = S - 512
    kk = np.arange(P)
    j = np.arange(4)
    qq = np.arange(512)
    mt = m[
        (base + qq)[None, None, :].repeat(P, 0).repeat(4, 1),
        (base + 128 * j[None, :, None] + kk[:, None, None]).repeat(512, 2),
    ]
    mb = np.where(mt != 0, np.float32(0.0), np.float32(-1e9)).astype(NP_MM)
    ident = np.eye(P, dtype=NP_MM)

    in_maps = []
    for c in range(8):
        b, g = c // 2, c % 2
        sl = slice(g * G, (g + 1) * G)
        in_maps.append(
            {
                "xqT": _arr_x(np.asarray(q[b]), S),
                "xkT": _arr_x(np.asarray(k[b]), S),
                "xvT": _arr_x(np.asarray(v[b]), S),
                "wqT": _arr_w(np.asarray(wq)[sl, :].T),
                "wkT": _arr_w(np.asarray(wk)[sl, :].T),
                "wvT": _arr_w(np.asarray(wv)[sl, :].T),
                "woT": _arr_w(np.asarray(wo)[:, sl].T),
                "bq": (np.asarray(bq)[sl] * 0.125).astype(f),
                "bk": np.asarray(bk)[sl].astype(f),
                "bv": np.asarray(bv)[sl].astype(NP_MM),
                "maskb": mb,
                "ident": ident,
            }
        )
    return in_maps


def kernel(q, k, v, mask, wq, bq, wk, bk, wv, bv, wo, bo):
    global LAST_RESULTS
    nc = build_program(S_FULL)
    in_maps = make_core_inputs(q, k, v, mask, wq, bq, wk, bk, wv, bv, wo, bo)
    res = run_bass_kernel_spmd(nc, in_maps, core_ids=list(range(8)), **RUN_KWARGS)
    LAST_RESULTS = res
    out = np.empty((B, S_FULL, D), np.float32)
    bo32 = np.asarray(bo, np.float32)
    for b in range(B):
        out[b] = res.results[2 * b]["yp"] + res.results[2 * b + 1]["yp"] + bo32
    return out


# revision 36
# speedup vs baseline: 1.0303x; 1.0061x over previous
"""Trainium2 Bass kernel for 16-head causal MHA (B=4, S=2048, D=1024).

Sharding: 8 cores = 4 batches x 2 head-groups (8 heads each).
Each core computes, for its batch b and head-group g:
  qh_T = (Wq_g @ x_q^T + bq_g) / 8   [512 j, S]   (j = head-major features)
  kh_T =  Wk_g @ x_k^T + bk_g        [512 j, S]
  vh   =  x_v @ Wv_g^T + bv_g        [S, 512 j]   (+ ones column per head)
  per head h (64-dim): block-causal scores_T [k, q], exp (no max
  subtraction; scores are O(1)), diagonal-block masking, ctx_T[dh,q]
  (+ sumexp row via the ones column), batched-reciprocal normalize,
  y_partial = ctx_T^T @ Wo_g^T       [S, 1024]
Host sums the two head-group partials per batch and adds bo.

Matmuls run in bf16 (fp32 PSUM accumulation); set MM_DT to float32r for
a higher-precision (but ~2.5x slower) variant.
"""

import os
import sys
from contextlib import ExitStack

sys.path.insert(0, "/opt/trn_rl_repo")

import numpy as np
import ml_dtypes

import concourse.bass as bass
import concourse.tile as tile
from concourse import mybir
from concourse.bass_utils import run_bass_kernel_spmd

F32 = mybir.dt.float32
MM_DT = mybir.dt.bfloat16
NP_MM = ml_dtypes.bfloat16
P = 128

# Full-problem dims
B, S_FULL, D, H = 4, 2048, 1024, 16
DH = D // H          # 64
G = 512              # features per head-group (8 heads x 64)
HLOC = 8             # heads per core

LAST_RESULTS = None  # set by kernel() for test harness introspection
RUN_KWARGS = {}  # extra kwargs for run_bass_kernel_spmd (test harness only)


def _mm(nc, out, lhsT, rhs, **kw):
    nc.tensor.matmul(out, lhsT, rhs, **kw)


_FIX_TYPES = {
    "InstMatmult",
    "InstActivation",
    "InstTensorCopy",
    "InstTensorTensor",
    "InstTensorScalarPtr",
    "InstTensorReduce",
    "InstReciprocal",
    "InstMemset",
    "InstDMACopy",
    "InstDrain",
    "InstEventSemaphore",
}


def _fixup_matmul_waits(nc):
    """walrus packs an instruction's sem waits into a single ISA struct with
    room for only ONE wait command (observed for Matmult/LW and Activation).
    Hoist excess waits onto same-engine nops inserted immediately before the
    offending instruction (same-engine order preserves the happens-before)."""
    import bass_rust as _br

    f = nc.m.functions[0]
    blocks = list(f.blocks)
    per_engine = {}
    for b in blocks:
        for inst in b.instructions:
            if type(inst).__name__ in _FIX_TYPES:
                si = inst.sync_info
                if si is not None and si.on_wait and len(si.on_wait) > 1:
                    eng = str(inst.engine)
                    per_engine[eng] = per_engine.get(eng, 0) + len(si.on_wait) - 1
    if not per_engine:
        return
    eng_map = {
        "EngineType.PE": nc.tensor,
        "EngineType.Activation": nc.scalar,
        "EngineType.DVE": nc.vector,
        "EngineType.Pool": nc.gpsimd,
        "EngineType.SP": nc.sync,
    }
    nop_stash = {}
    for eng, cnt in per_engine.items():
        lens_before = {b.name: len(b.instructions) for b in blocks}
        handle = eng_map[eng]
        for _ in range(cnt):
            handle.nop()
        endb = next(b for b in blocks if len(b.instructions) != lens_before[b.name])
        endlist = endb.instructions
        nop_stash[eng] = list(endlist[-cnt:])
        endb.instructions = endlist[:-cnt]
    for b in blocks:
        insts = b.instructions
        out = []
        changed = False
        for inst in insts:
            if type(inst).__name__ in _FIX_TYPES:
                si = inst.sync_info
                if si is not None and si.on_wait and len(si.on_wait) > 1:
                    waits = list(si.on_wait)
                    eng = str(inst.engine)
                    for w in waits[:-1]:
                        nop = nop_stash[eng].pop(0)
                        nop.sync_info = _br.SyncInfo(on_wait=[w], on_update=[])
                        out.append(nop)
                    si.on_wait = [waits[-1]]
                    changed = True
            out.append(inst)
        if changed:
            b.instructions = out


def build_program(S=S_FULL):
    """Build the single-core Bass program (same program on all 8 cores)."""
    ND = D // P           # 8 d-tiles
    NJ = G // P           # 4 j-tiles
    NST = S // P          # s-tiles
    NCH = S // 512        # q-chunks (and s-quarters)

    nc = bass.Bass("TRN2", target_bir_lowering=False, debug=False)

    xqT = nc.dram_tensor("xqT", [ND, NCH, P, 512], MM_DT, kind="ExternalInput").ap()
    xkT = nc.dram_tensor("xkT", [ND, NCH, P, 512], MM_DT, kind="ExternalInput").ap()
    xvT = nc.dram_tensor("xvT", [ND, NCH, P, 512], MM_DT, kind="ExternalInput").ap()
    wqT = nc.dram_tensor("wqT", [P, ND, G], MM_DT, kind="ExternalInput").ap()
    wkT = nc.dram_tensor("wkT", [P, ND, G], MM_DT, kind="ExternalInput").ap()
    wvT = nc.dram_tensor("wvT", [P, ND, G], MM_DT, kind="ExternalInput").ap()
    woT = nc.dram_tensor("woT", [P, NJ, D], MM_DT, kind="ExternalInput").ap()
    bqd = nc.dram_tensor("bq", [G], F32, kind="ExternalInput").ap()
    bkd = nc.dram_tensor("bk", [G], F32, kind="ExternalInput").ap()
    bvd = nc.dram_tensor("bv", [G], MM_DT, kind="ExternalInput").ap()
    mkd = nc.dram_tensor("maskb", [P, 4, 512], MM_DT, kind="ExternalInput").ap()
    idd = nc.dram_tensor("ident", [P, P], MM_DT, kind="ExternalInput").ap()
    yp = nc.dram_tensor("yp", [S, D], F32, kind="ExternalOutput").ap()

    with tile.TileContext(nc) as tc, ExitStack() as ctx:
        consts = ctx.enter_context(tc.tile_pool(name="consts", bufs=1))
        big = ctx.enter_context(tc.tile_pool(name="big", bufs=1))
        wpool = ctx.enter_context(tc.tile_pool(name="weights", bufs=4))
        xpool = ctx.enter_context(tc.tile_pool(name="xt", bufs=48))
        apool = ctx.enter_context(tc.tile_pool(name="attn", bufs=4))
        opool = ctx.enter_context(tc.tile_pool(name="outev", bufs=2))
        rpool = ctx.enter_context(tc.tile_pool(name="recip", bufs=2))
        pp = ctx.enter_context(tc.tile_pool(name="pp", bufs=2, space="PSUM"))
        sp = ctx.enter_context(tc.tile_pool(name="sp", bufs=2, space="PSUM"))
        cp = ctx.enter_context(tc.tile_pool(name="cp", bufs=2, space="PSUM"))

        # --- constants ---
        ones1f = consts.tile([1, P], F32)
        nc.vector.memset(ones1f, 1.0)
        ones1 = consts.tile([1, P], MM_DT)
        nc.vector.tensor_copy(ones1, ones1f)
        bvrow = consts.tile([1, G], MM_DT)
        nc.sync.dma_start(bvrow, bvd.rearrange("(o g) -> o g", o=1))
        bqc = consts.tile([P, NJ], F32)
        nc.sync.dma_start(bqc, bqd.rearrange("(a p) -> p a", a=NJ))
        bkc = consts.tile([P, NJ], F32)
        nc.sync.dma_start(bkc, bkd.rearrange("(a p) -> p a", a=NJ))
        maskb = consts.tile([P, 4, 512], MM_DT)
        nc.sync.dma_start(maskb, mkd)
        ident = consts.tile([P, P], MM_DT)
        nc.sync.dma_start(ident, idd)

        # --- persistent activations ---
        QT = big.tile([P, NJ, S], MM_DT)
        KT = big.tile([P, NJ, S], MM_DT)
        CT = big.tile([P, NJ, S], MM_DT)   # normalized ctx_T for the out-proj
        V = big.tile([P, NST, HLOC, DH + 1], MM_DT)
        onesvf = consts.tile([P, NST * HLOC], F32)
        nc.vector.memset(onesvf, 1.0)
        nc.vector.tensor_copy(V[:, :, :, DH : DH + 1], onesvf)

        # --- weights (wv first: v-proj runs first; rest after v-proj x DMAs) ---
        wv_sb = wpool.tile([P, ND, G], MM_DT, tag="w")
        nc.sync.dma_start(wv_sb, wvT)

        # --- q/k projections, jt-at-a-time so attention can interleave ---
        def load_x(x_dram):
            xs = {}
            for sq in range(NCH):
                for dt in range(ND):
                    xt = xpool.tile([P, 512], MM_DT, tag="x")
                    nc.scalar.dma_start(xt, x_dram[dt, sq])
                    xs[dt, sq] = xt
            return xs

        def proj_jt(xs, w_sb, dst, bias_sb, scale, jt):
            for sq in range(NCH):
                ps = pp.tile([P, 512], F32, tag="pp")
                for dt in range(ND):
                    _mm(
                        nc, ps,
                        w_sb[:, dt, jt * P : (jt + 1) * P],
                        xs[dt, sq],
                        start=(dt == 0), stop=(dt == ND - 1),
                    )
                nc.vector.tensor_scalar(
                    dst[:, jt, sq * 512 : (sq + 1) * 512], ps,
                    scale, bias_sb[:, jt : jt + 1],
                    mybir.AluOpType.mult, mybir.AluOpType.add,
                )

        # --- v projection first: out [s-part, j-free], ones col per head ---
        for sq in range(NCH):
            xs = []
            for dt in range(ND):
                xt = xpool.tile([P, 512], MM_DT, tag="x")
                nc.scalar.dma_start(xt, xvT[dt, sq])
                xs.append(xt)
            for stl in range(4):
                st = sq * 4 + stl
                ps = pp.tile([P, G], F32, tag="pp")
                for dt in range(ND):
                    _mm(
                        nc, ps,
                        xs[dt][:, stl * P : (stl + 1) * P],
                        wv_sb[:, dt, :],
                        start=(dt == 0), stop=False,
                    )
                _mm(nc, ps, ones1, bvrow, start=False, stop=True)
                nc.vector.tensor_copy(
                    V[:, st, :, 0:DH], ps.rearrange("p (h d) -> p h d", h=HLOC)
                )
            if sq == 0:
                wk_sb = wpool.tile([P, ND, G], MM_DT, tag="w")
                nc.sync.dma_start(wk_sb, wkT)
                wq_sb = wpool.tile([P, ND, G], MM_DT, tag="w")
                nc.sync.dma_start(wq_sb, wqT)
                wo_sb = wpool.tile([P, NJ, D], MM_DT, tag="w")
                nc.sync.dma_start(wo_sb, woT)

        xs_k = load_x(xkT)
        for jt in range(NJ):
            proj_jt(xs_k, wk_sb, KT, bkc, 1.0, jt)
        xs_q = load_x(xqT)
        proj_jt(xs_q, wq_sb, QT, bqc, 0.125, 0)

        # --- attention: head pairs interleaved at chunk level ---
        pending_norm = None
        for t in range(HLOC // 2):
            heads = (2 * t, 2 * t + 1)
            sums = {}
            for h in heads:
                sums_t = rpool.tile([P, 512], F32, tag=f"sums{h % 2}")
                nc.vector.memset(sums_t, 1.0)
                sums[h] = sums_t
            for c in range(NCH):
                cs = slice(c * 512, (c + 1) * 512)
                hA, hB = heads
                jtp = t
                ctxA = cp.tile([P, 512], F32, tag="cp")
                ctxB = cp.tile([P, 512], F32, tag="cp")
                nkt = 4 * (c + 1)
                for kt in range(nkt):
                    diag = kt >= nkt - 4
                    scp = sp.tile([P, 2, 512], F32, tag="sp")
                    _mm(
                        nc, scp[:, 0, :],
                        KT[0:DH, jtp, kt * P : (kt + 1) * P],
                        QT[0:DH, jtp, cs],
                        start=True, stop=not diag,
                    )
                    _mm(
                        nc, scp[:, 1, :],
                        KT[DH:P, jtp, kt * P : (kt + 1) * P],
                        QT[DH:P, jtp, cs],
                        start=True, stop=not diag,
                    )
                    if diag:  # accumulate -1e9 above the diagonal, pre-exp
                        moff = kt - (nkt - 4)
                        _mm(
                            nc, scp[:, 0, :], ident, maskb[:, moff, :],
                            start=False, stop=True,
                        )
                        _mm(
                            nc, scp[:, 1, :], ident, maskb[:, moff, :],
                            start=False, stop=True,
                        )
                    at = apool.tile([P, 2, 512], MM_DT, tag="attn")
                    nc.scalar.activation(
                        at, scp, mybir.ActivationFunctionType.Exp
                    )
                    _mm(
                        nc, ctxA[0 : DH + 1, :],
                        V[:, kt, hA, :], at[:, 0, :],
                        start=(kt == 0), stop=(kt == nkt - 1),
                    )
                    _mm(
                        nc, ctxB[0 : DH + 1, :],
                        V[:, kt, hB, :], at[:, 1, :],
                        start=(kt == 0), stop=(kt == nkt - 1),
                    )
                # evict unnormalized ctx into CT; stash the sumexp rows
                nc.vector.tensor_copy(CT[0:DH, jtp, cs], ctxA[0:DH, :])
                nc.vector.tensor_copy(
                    sums[hA][32 * c : 32 * c + 1, :], ctxA[DH : DH + 1, :]
                )
                nc.vector.tensor_copy(CT[DH:P, jtp, cs], ctxB[0:DH, :])
                nc.vector.tensor_copy(
                    sums[hB][32 * c : 32 * c + 1, :], ctxB[DH : DH + 1, :]
                )
                if c == min(2, NCH - 1) and pending_norm is not None:
                    pending_norm()
                    pending_norm = None
            if t + 1 < NJ:
                proj_jt(xs_q, wq_sb, QT, bqc, 0.125, t + 1)
            # start DVE reciprocal chains now; defer broadcast+mul to next pair
            norm_parts = []
            for h in heads:
                jt, half = h // 2, (h % 2) * DH
                rec_h = rpool.tile([P, 512], F32, tag=f"rech{h % 2}")
                nc.vector.reciprocal(rec_h, sums[h])
                rstages = []
                for c in range(NCH):
                    rstage = rpool.tile([1, 512], MM_DT, tag=f"rst{h % 2}_{c}")
                    nc.vector.tensor_copy(rstage, rec_h[32 * c : 32 * c + 1, :])
                    rstages.append(rstage)
                norm_parts.append((jt, half, rstages))

            def _make_norm(parts=norm_parts):
                def _norm():
                    for jt, half, rstages in parts:
                        for c in range(NCH):
                            cs = slice(c * 512, (c + 1) * 512)
                            rbp = pp.tile([P, 512], F32, tag="pp")
                            _mm(
                                nc, rbp[0:DH, :], ones1[:, 0:DH], rstages[c],
                                start=True, stop=True,
                            )
                            rb = rpool.tile([P, 512], MM_DT, tag="rb")
                            nc.vector.tensor_copy(
                                rb[half : half + DH, :], rbp[0:DH, :]
                            )
                            nc.vector.tensor_mul(
                                CT[half : half + DH, jt, cs],
                                CT[half : half + DH, jt, cs],
                                rb[half : half + DH, :],
                            )
                return _norm

            pending_norm = _make_norm()
        if pending_norm is not None:
            pending_norm()
            pending_norm = None

        # --- output projection: y[s, e] = ctx_T^T @ woT ---
        for st in range(NST):
            ob = opool.tile([P, D], F32, tag="ob")
            for ec in range(2):
                ps = pp.tile([P, 512], F32, tag="pp")
                for jtt in range(NJ):
                    _mm(
                        nc, ps,
                        CT[:, jtt, st * P : (st + 1) * P],
                        wo_sb[:, jtt, ec * 512 : (ec + 1) * 512],
                        start=(jtt == 0), stop=(jtt == NJ - 1),
                    )
                nc.vector.tensor_copy(ob[:, ec * 512 : (ec + 1) * 512], ps)
            nc.sync.dma_start(yp[st * P : (st + 1) * P, :], ob)

    _fixup_matmul_waits(nc)
    return nc


def _arr_x(x, S):
    """[S, D] -> [ND, NCH, P, 512] bf16, contiguous DMA tiles of x^T."""
    xT = x.T.astype(NP_MM)  # [D, S]
    return np.ascontiguousarray(
        xT.reshape(D // P, P, S // 512, 512).transpose(0, 2, 1, 3)
    )


def _arr_w(wT):
    """[K, N] (K mult of 128) -> [P, K//P, N] bf16 partition-major tiles."""
    wT = np.asarray(wT).astype(NP_MM)
    kk, n = wT.shape
    return np.ascontiguousarray(wT.reshape(kk // P, P, n).transpose(1, 0, 2))


def make_core_inputs(q, k, v, mask, wq, bq, wk, bk, wv, bv, wo, bo, S=S_FULL):
    """Build the 8 per-core input dicts. Core c -> batch c//2, head-group c%2."""
    f = np.float32
    # block-causal diagonal mask tiles [kk, j, qq] = mask[q0+qq, q0+128j+kk]
    m = np.asarray(mask)[0, 0]
    base = S - 512
    kk = np.arange(P)
    j = np.arange(4)
    qq = np.arange(512)
    mt = m[
        (base + qq)[None, None, :].repeat(P, 0).repeat(4, 1),
        (base + 128 * j[None, :, None] + kk[:, None, None]).repeat(512, 2),
    ]
    mb = np.where(mt != 0, np.float32(0.0), np.float32(-1e9)).astype(NP_MM)
    ident = np.eye(P, dtype=NP_MM)

    in_maps = []
    for c in range(8):
        b, g = c // 2, c % 2
        sl = slice(g * G, (g + 1) * G)
        in_maps.append(
            {
                "xqT": _arr_x(np.asarray(q[b]), S),
                "xkT": _arr_x(np.asarray(k[b]), S),
                "xvT": _arr_x(np.asarray(v[b]), S),
                "wqT": _arr_w(np.asarray(wq)[sl, :].T),
                "wkT": _arr_w(np.asarray(wk)[sl, :].T),
                "wvT": _arr_w(np.asarray(wv)[sl, :].T),
                "woT": _arr_w(np.asarray(wo)[:, sl].T),
                "bq": (np.asarray(bq)[sl] * 0.125).astype(f),
                "bk": np.asarray(bk)[sl].astype(f),
                "bv": np.asarray(bv)[sl].astype(NP_MM),
                "maskb": mb,
                "ident": ident,
            }
        )
    return in_maps


def kernel(q, k, v, mask, wq, bq, wk, bk, wv, bv, wo, bo):
    global LAST_RESULTS
    nc = build_program(S_FULL)
    in_maps = make_core_inputs(q, k, v, mask, wq, bq, wk, bk, wv, bv, wo, bo)
    res = run_bass_kernel_spmd(nc, in_maps, core_ids=list(range(8)), **RUN_KWARGS)
    LAST_RESULTS = res
    out = np.empty((B, S_FULL, D), np.float32)
    bo32 = np.asarray(bo, np.float32)
    for b in range(B):
        out[b] = res.results[2 * b]["yp"] + res.results[2 * b + 1]["yp"] + bo32
    return out
